# revision 20
# baseline (speedup 1.0000x reference)
"""Trainium2 Bass kernel for nn_BottomLevelDecoderRNN.

2-layer GRU decoder, H=1024, S=16 steps, E*B = 2048 independent sequences,
data-parallel over 8 NeuronCores (R = 256 rows per core), everything kept
transposed as [feature, row].

Per-step math (per core):
  GRU1 r/z: ps = DR(wp8, pv8) + DR(w1h_rz, h1_8)        [fp8 DoubleRow, x256]
            + cached_rz (DVE; bih1+bhh1 folded)  -> sigmoid(ps/256)
  GRU1 n:   psh = w1h_n@h1 (fp16) -> ghb;  psg = wp_n@pv + cached_n + r*ghb
            -> tanh -> h1' = n + z*(h1-n)  (batched DVE) -> fp8 copy
  GRU2 r/z: ps = DR(w2h_rz, h2_8) + DR(w2i_rz, h1'_8) -> sigmoid(ps/256 + b)
  GRU2 n:   psh = DR(w2h_n, h2_8) -> ghb2; psg = w2i_n@h1' (fp16) + r*ghb2
            -> tanh.  fco output GEMM in fp16.

fp8 path: weights scaled x256 into e4m3 (clip 240), h states / prev cast to
e4m3 unscaled; PSUM carries 256x pre-activations, descaled by the
activation's scale=1/256. w1h/w2i n-gate GEMMs (error-sensitive) stay fp16.
All weights SBUF-resident. Emulated rel-err 9.8e-3 (gate 2e-2).
"""
import numpy as np

E, B, C, H, D = 16, 128, 512, 1024, 130
S = 16
NCORES = 8
EPC = E // NCORES        # 2 embeddings per core
R = EPC * B              # 256 rows per core
KH = H // 128            # 8 h k-tiles
KP = KH // 2             # 4 DR k-pairs
MG = 3 * H // 128        # 24 gate m-tiles
MRZ = 2 * H // 128       # 16 rz m-tiles
NJ = H // 128            # 8 n/h tiles
KC = C // 128            # 4 c k-tiles
MI = 2 * H // 128        # 16 init m-tiles
WS = 256.0               # fp8 weight scale

# bias tile column layout ([128, NBIAS] fp32)
B_INIT = 0      # 16: fc_init_b
B_N1H = 32      # 8:  bhh1[2H:]
B_IH1 = 40      # 24: rz: (bih1+bhh1)*256;  n: bih1
B_RZ2 = 64      # 16: bih2[:2H]+bhh2[:2H]
B_N2H = 80      # 8:  bhh2[2H:]
B_N2I = 88      # 8:  bih2[2H:]
B_FCO = 96      # 2:  fco_b
NBIAS = 98

_cache = {}


def _wtiles(w_t, nm, nk):
    """[K, M] (w_t = W.T) -> [nm, 128, nk*128] fp16 stationary chunks."""
    Kf, Mf = w_t.shape
    assert Kf == nk * 128 and Mf == nm * 128
    return np.ascontiguousarray(
        w_t.reshape(nk, 128, nm, 128).transpose(2, 1, 0, 3).reshape(nm, 128, nk * 128)
    ).astype(np.float16)


def _q8(x):
    import ml_dtypes
    return np.clip(x, -240, 240).astype(ml_dtypes.float8_e4m3)


def _wtiles_dr(w, nm, scale=WS):
    """[nm*128, H] weight part -> [nm, 128, KP*2*128] fp8e4 DoubleRow chunks:
    chunk[m][p, kt, j, c] = (W.T)[kt*256 + j*128 + p, m*128 + c] * scale."""
    wt = np.asarray(w, np.float32).T * scale          # [H, nm*128]
    arr = wt.reshape(KP, 2, 128, nm, 128).transpose(3, 2, 0, 1, 4)
    return np.ascontiguousarray(_q8(arr).reshape(nm, 128, KP * 2 * 128))


def _bias_cols(vec, n):
    return np.ascontiguousarray(vec.reshape(n, 128).T).astype(np.float32)


def build_program():
    import concourse.tile as tile
    from concourse import bacc, mybir

    f32, f16, f8 = mybir.dt.float32, mybir.dt.float16, mybir.dt.float8e4
    Sig = mybir.ActivationFunctionType.Sigmoid
    Tanh = mybir.ActivationFunctionType.Tanh
    Ident = mybir.ActivationFunctionType.Identity
    DRow = mybir.MatmulPerfMode.DoubleRow

    nc = bacc.Bacc("TRN2", target_bir_lowering=False, debug=False,
                   enable_asserts=False, num_devices=NCORES)

    def din(name, shape, dt=f16):
        return nc.dram_tensor(name, shape, dt, kind="ExternalInput").ap()

    cflatT = din("cflatT", [KC, 128, R])
    prevT0 = din("prevT0", [S, 128, R])
    prevT1 = din("prevT1", [S, 128, R])
    prevT8 = din("prevT8", [S, 128, 2, R], f8)
    w1h8 = din("w1h8", [MRZ // 4, 128, 4 * KP * 2 * 128], f8)
    w2i8 = din("w2i8", [MRZ // 4, 128, 4 * KP * 2 * 128], f8)
    w2h8 = din("w2h8", [MRZ // 4, 128, 4 * KP * 2 * 128], f8)
    w2hn8 = din("w2hn8", [NJ // 4, 128, 4 * KP * 2 * 128], f8)
    w1hn = din("w1hn", [NJ // 4, 128, 4 * KH * 128])
    w2in = din("w2in", [NJ // 4, 128, 4 * KH * 128])
    wp8 = din("wp8", [128, MRZ, 2, 128], f8)
    wp0n = din("wp0n", [128, NJ * 128])
    wp1n = din("wp1n", [128, NJ * 128])
    wc = din("wc", [6, 128, 4 * KC * 128])      # 4 m-tiles per chunk
    wini = din("wini", [4, 128, 4 * KC * 128])
    wfco = din("wfco", [128, KH * 256])
    biases = din("biases", [128, NBIAS], f32)
    yT = nc.dram_tensor("yT", [S, 132, R], f32, kind="ExternalOutput").ap()

    with tile.TileContext(nc) as tc:
        with tc.tile_pool(name="const", bufs=1) as const, \
             tc.tile_pool(name="stream", bufs=4) as stream, \
             tc.tile_pool(name="state", bufs=2) as state, \
             tc.tile_pool(name="gates", bufs=2) as gates, \
             tc.tile_pool(name="tmp", bufs=2) as tmp, \
             tc.tile_pool(name="prevp", bufs=2) as prevp, \
             tc.tile_pool(name="outp", bufs=2) as outp, \
             tc.tile_pool(name="psA", bufs=3, space="PSUM") as psA, \
             tc.tile_pool(name="psB", bufs=5, space="PSUM") as psB:

            # ---- small constant loads ----
            bias_sb = const.tile([128, NBIAS], f32, tag="bias")
            nc.sync.dma_start(bias_sb[:], biases[:])
            cfl_sb = const.tile([128, KC * R], f16, tag="cfl")
            for k in range(KC):
                nc.sync.dma_start(cfl_sb[:, k * R:(k + 1) * R], cflatT[k])

            def bias_ap(col):
                return bias_sb[:, col:col + 1]

            # ---- resident weights (scalar-issuer queue, interleaved with
            # init compute; ordered by first use) ----
            w1h8_sb = const.tile([128, MRZ, KP, 2, 128], f8, tag="w1h8")
            w2i8_sb = const.tile([128, MRZ, KP, 2, 128], f8, tag="w2i8")
            w2h8_sb = const.tile([128, MRZ, KP, 2, 128], f8, tag="w2h8")
            w2hn8_sb = const.tile([128, NJ, KP, 2, 128], f8, tag="w2hn8")
            w1hn_sb = const.tile([128, NJ, KH * 128], f16, tag="w1hn")
            w2in_sb = const.tile([128, NJ, KH * 128], f16, tag="w2in")
            wp8_sb = const.tile([128, MRZ, 2, 128], f8, tag="wp8")
            wp0n_sb = const.tile([128, NJ * 128], f16, tag="wp0n")
            wp1n_sb = const.tile([128, NJ * 128], f16, tag="wp1n")
            wfco_sb = const.tile([128, KH * 256], f16, tag="wfco")

            def weight_feed():
                # only what step 0's first phases need pre-loop; the rest is
                # issued inside step 0 right before first use (DMA deps are
                # program-order counting semaphores).
                yield lambda: nc.scalar.dma_start(wp8_sb[:], wp8[:])
                for g in range(MRZ // 4):
                    yield lambda g=g: nc.scalar.dma_start(
                        w1h8_sb[:, 4 * g:4 * g + 4], w1h8[g])
                for g in range(NJ // 4):
                    yield lambda g=g: nc.scalar.dma_start(
                        w1hn_sb[:, 4 * g:4 * g + 4], w1hn[g])
                yield lambda: nc.scalar.dma_start(wp0n_sb[:], wp0n[:])
                yield lambda: nc.scalar.dma_start(wp1n_sb[:], wp1n[:])
            wfeed = weight_feed()

            def feed(n=1):
                for _ in range(n):
                    f = next(wfeed, None)
                    if f is not None:
                        f()

            # ---- h init: t0T = tanh(wini @ cflatT + binit) ----
            h1T = state.tile([128, NJ, R], f16, tag="h1")
            h2T = state.tile([128, NJ, R], f16, tag="h2")
            h18 = state.tile([128, NJ, R], f8, tag="h18")
            h28 = state.tile([128, NJ, R], f8, tag="h28")
            for g in range(4):
                wchunk = stream.tile([128, 4 * KC * 128], f16, tag="stream")
                nc.sync.dma_start(wchunk[:], wini[g])
                feed(3)
                for mi in range(4):
                    m = 4 * g + mi
                    ps = psB.tile([128, R], f32, tag="g")
                    for k in range(KC):
                        nc.tensor.matmul(
                            ps[:], wchunk[:, (mi * KC + k) * 128:(mi * KC + k + 1) * 128],
                            cfl_sb[:, k * R:(k + 1) * R],
                            start=(k == 0), stop=(k == KC - 1))
                    dst = h1T if m < NJ else h2T
                    nc.scalar.activation(dst[:, m % NJ], ps[:], Tanh,
                                         bias=bias_ap(B_INIT + m))
            for j in range(NJ):
                nc.vector.tensor_copy(h18[:, j], h1T[:, j])
                nc.vector.tensor_copy(h28[:, j], h2T[:, j])

            # ---- cached = Wc @ cflatT + biases (rz part scaled x256) ----
            cached_sb = const.tile([128, MG, R], f16, tag="cached")
            for g in range(6):
                wchunk = stream.tile([128, 4 * KC * 128], f16, tag="stream")
                nc.sync.dma_start(wchunk[:], wc[g])
                feed(3)
                for mi in range(4):
                    m = 4 * g + mi
                    ps = psB.tile([128, R], f32, tag="g")
                    for k in range(KC):
                        nc.tensor.matmul(
                            ps[:], wchunk[:, (mi * KC + k) * 128:(mi * KC + k + 1) * 128],
                            cfl_sb[:, k * R:(k + 1) * R],
                            start=(k == 0), stop=(k == KC - 1))
                    sc = WS if m < MRZ else 1.0
                    nc.scalar.activation(cached_sb[:, m], ps[:], Ident,
                                         bias=bias_ap(B_IH1 + m), scale=sc)
            feed(9)

            def fco_step(h2T_cur, s):
                for mo, msz, osz, bc in [(0, 128, 128, B_FCO), (128, 32, 2, B_FCO + 1)]:
                    ps = psB.tile([128, R], f32, tag="g")
                    for k in range(KH):
                        nc.tensor.matmul(ps[0:msz, :],
                                         wfco_sb[:, k * 256 + mo: k * 256 + mo + msz],
                                         h2T_cur[:, k],
                                         start=(k == 0), stop=(k == KH - 1))
                    ysb = outp.tile([128, R], f32, tag="y")
                    nc.scalar.activation(ysb[0:osz, :], ps[0:osz, :], Ident,
                                         bias=bias_sb[0:osz, bc:bc + 1])
                    nc.sync.dma_start(yT[s, mo:mo + osz, :], ysb[0:osz, :])

            h2T_done = []  # (h2T tile, step) pending fco

            for s in range(S):
                pv8 = prevp.tile([128, 2, R], f8, tag="pv8")
                nc.sync.dma_start(pv8[:], prevT8[s])
                pv0 = prevp.tile([128, R], f16, tag="pv0")
                nc.sync.dma_start(pv0[:], prevT0[s])
                pv1 = prevp.tile([128, R], f16, tag="pv1")
                nc.sync.dma_start(pv1[:], prevT1[s])
                if s == 0:
                    qs = [nc.sync, nc.scalar]
                    for g in range(NJ // 4):
                        qs[g % 2].dma_start(w2hn8_sb[:, 4 * g:4 * g + 4], w2hn8[g])
                    for g in range(MRZ // 4):
                        qs[(g + 1) % 2].dma_start(w2h8_sb[:, 4 * g:4 * g + 4], w2h8[g])

                # ---------- GRU1 r/z (all fp8 DR, x256 PSUM; m-tile pairs
                # share a [128,2,R] PSUM tile and one ACT; biases folded
                # into cached) ----------
                # rz pairs interleaved with GRU1-n gh groups so the scalar
                # queue drains PSUM evenly (no ACT backlog at phase end)
                r1 = gates.tile([128, NJ, R], f16, tag="rg")
                z1 = gates.tile([128, NJ, R], f16, tag="zg")
                ghb1 = tmp.tile([128, NJ, R], f16, tag="ghb")
                tt1 = tmp.tile([128, NJ, R], f16, tag="tt")
                for p in range(MRZ // 2):
                    ps = psA.tile([128, 2, R], f32, tag="rz")
                    for mi in range(2):
                        m = 2 * p + mi
                        nc.tensor.matmul(ps[:, mi], wp8_sb[:, m], pv8[:],
                                         start=True, stop=False, perf_mode=DRow)
                        for kt in range(KP):
                            nc.tensor.matmul(ps[:, mi], w1h8_sb[:, m, kt],
                                             h18[:, 2 * kt:2 * kt + 2, :],
                                             start=False, stop=(kt == KP - 1),
                                             perf_mode=DRow)
                    nc.vector.tensor_add(ps[:], ps[:], cached_sb[:, 2 * p:2 * p + 2])
                    dst = r1 if p < NJ // 2 else z1
                    jj = (2 * p) % NJ
                    nc.scalar.activation(dst[:, jj:jj + 2], ps[:], Sig,
                                         bias=0.0, scale=1.0 / WS)
                    j = p
                    psh = psB.tile([128, R], f32, tag="g")
                    for k in range(KH):
                        nc.tensor.matmul(psh[:], w1hn_sb[:, j, k * 128:(k + 1) * 128],
                                         h1T[:, k],
                                         start=(k == 0), stop=(k == KH - 1))
                    nc.scalar.activation(ghb1[:, j], psh[:], Ident,
                                         bias=bias_ap(B_N1H + j))
                    if j % 4 == 3:
                        hs = slice(j - 3, j + 1)
                        nc.vector.tensor_mul(tt1[:, hs], r1[:, hs], ghb1[:, hs])
                        nc.vector.tensor_add(tt1[:, hs], tt1[:, hs],
                                             cached_sb[:, MRZ + j - 3:MRZ + j + 1])
                    if s == 0 and p % 2 == 1:
                        g = p // 2
                        (nc.sync if g % 2 else nc.scalar).dma_start(
                            w2i8_sb[:, 4 * g:4 * g + 4], w2i8[g])
                n1 = gates.tile([128, NJ, R], f16, tag="ng")
                for j in range(NJ):
                    psg = psB.tile([128, R], f32, tag="g")
                    nc.tensor.matmul(psg[:], wp0n_sb[:, j * 128:(j + 1) * 128], pv0[:],
                                     start=True, stop=False)
                    nc.tensor.matmul(psg[:], wp1n_sb[:, j * 128:(j + 1) * 128], pv1[:],
                                     start=False, stop=True)
                    nc.vector.tensor_add(psg[:], psg[:], tt1[:, j])
                    nc.scalar.activation(n1[:, j], psg[:], Tanh, bias=0.0)
                # h1' = n1 + z1*(h1 - n1) in j-halves, fp8 copies right after
                d1 = tmp.tile([128, NJ, R], f16, tag="tt")
                h1T_new = state.tile([128, NJ, R], f16, tag="h1")
                h18_new = state.tile([128, NJ, R], f8, tag="h18")
                for hf in range(2):
                    hs = slice(4 * hf, 4 * hf + 4)
                    nc.vector.tensor_sub(d1[:, hs], h1T[:, hs], n1[:, hs])
                    nc.vector.tensor_mul(d1[:, hs], z1[:, hs], d1[:, hs])
                    nc.vector.tensor_add(h1T_new[:, hs], n1[:, hs], d1[:, hs])
                    for j in range(4 * hf, 4 * hf + 4):
                        nc.vector.tensor_copy(h18_new[:, j], h1T_new[:, j])

                if s == 0:
                    qs = [nc.sync, nc.scalar]
                    for g in range(NJ // 4):
                        qs[g % 2].dma_start(w2in_sb[:, 4 * g:4 * g + 4], w2in[g])
                    nc.scalar.dma_start(wfco_sb[:], wfco[:])

                # fco for previous step (PE filler over the h1 update chain)
                if h2T_done:
                    fco_step(*h2T_done.pop())

                # ---------- GRU2 n gh-part (fp8 DR on old h2) ----------
                ghb2 = tmp.tile([128, NJ, R], f16, tag="ghb")
                for j in range(NJ):
                    psh = psB.tile([128, R], f32, tag="g")
                    for kt in range(KP):
                        nc.tensor.matmul(psh[:], w2hn8_sb[:, j, kt],
                                         h28[:, 2 * kt:2 * kt + 2, :],
                                         start=(kt == 0), stop=(kt == KP - 1),
                                         perf_mode=DRow)
                    nc.scalar.activation(ghb2[:, j], psh[:], Ident,
                                         bias=bias_ap(B_N2H + j), scale=1.0 / WS)

                # ---------- GRU2 r/z (all fp8 DR) ----------
                r2 = gates.tile([128, NJ, R], f16, tag="rg")
                z2 = gates.tile([128, NJ, R], f16, tag="zg")
                tt2 = tmp.tile([128, NJ, R], f16, tag="tt")
                n2 = gates.tile([128, NJ, R], f16, tag="ng")

                def psg2_group(j):
                    psg = psB.tile([128, R], f32, tag="g")
                    for k in range(KH):
                        nc.tensor.matmul(psg[:], w2in_sb[:, j, k * 128:(k + 1) * 128],
                                         h1T_new[:, k],
                                         start=(k == 0), stop=(k == KH - 1))
                    nc.vector.tensor_add(psg[:], psg[:], tt2[:, j])
                    nc.scalar.activation(n2[:, j], psg[:], Tanh,
                                         bias=bias_ap(B_N2I + j))

                # rz pairs; from m>=8 interleave the GRU2-n psg groups so
                # the scalar queue drains PSUM evenly at phase end
                for m in range(MRZ):
                    ps = psB.tile([128, R], f32, tag="g")
                    for kt in range(KP):
                        nc.tensor.matmul(ps[:], w2h8_sb[:, m, kt],
                                         h28[:, 2 * kt:2 * kt + 2, :],
                                         start=(kt == 0), stop=False,
                                         perf_mode=DRow)
                    for kt in range(KP):
                        nc.tensor.matmul(ps[:], w2i8_sb[:, m, kt],
                                         h18_new[:, 2 * kt:2 * kt + 2, :],
                                         start=False, stop=(kt == KP - 1),
                                         perf_mode=DRow)
                    dst = r2 if m < NJ else z2
                    nc.scalar.activation(dst[:, m % NJ], ps[:], Sig,
                                         bias=bias_ap(B_RZ2 + m), scale=1.0 / WS)
                    if m == 3 or m == 7:
                        hs = slice(m - 3, m + 1)
                        nc.vector.tensor_mul(tt2[:, hs], r2[:, hs], ghb2[:, hs])
                    if m >= NJ:
                        psg2_group(m - NJ)
                d2 = tmp.tile([128, NJ, R], f16, tag="tt")
                nc.vector.tensor_sub(d2[:], h2T[:], n2[:])
                nc.vector.tensor_mul(d2[:], z2[:], d2[:])
                h2T_new = state.tile([128, NJ, R], f16, tag="h2")
                nc.vector.tensor_add(h2T_new[:], n2[:], d2[:])
                h28_new = state.tile([128, NJ, R], f8, tag="h28")
                nc.gpsimd.tensor_copy(h28_new[:], h2T_new[:])

                h1T, h2T = h1T_new, h2T_new
                h18, h28 = h18_new, h28_new
                h2T_done.append((h2T, s))

            fco_step(*h2T_done.pop())

    nc.compile()
    return nc


def prep_inputs(c, target, fc_init_w, fc_init_b, g1_wih, g1_whh, g1_bih, g1_bhh,
                g2_wih, g2_whh, g2_bih, g2_bhh, fco_w, fco_b):
    """Host-side shard/layout prep. Returns per-core input maps."""
    f16 = np.float16
    T = E * S
    c = np.asarray(c, np.float32)
    target = np.asarray(target, np.float32)

    g1_wih = np.asarray(g1_wih, np.float32)
    g1_whh = np.asarray(g1_whh, np.float32)
    g2_wih = np.asarray(g2_wih, np.float32)
    g2_whh = np.asarray(g2_whh, np.float32)
    def _g4(tiles):
        nm, _, w = tiles.shape
        return np.ascontiguousarray(
            tiles.reshape(nm // 4, 4, 128, w).transpose(0, 2, 1, 3)
            .reshape(nm // 4, 128, 4 * w))

    w1h8_a = _g4(_wtiles_dr(g1_whh[:2 * H], MRZ))
    w2i8_a = _g4(_wtiles_dr(g2_wih[:2 * H], MRZ))
    w2h8_a = _g4(_wtiles_dr(g2_whh[:2 * H], MRZ))
    w2hn8_a = _g4(_wtiles_dr(g2_whh[2 * H:], NJ))
    w1hn_a = _g4(_wtiles(np.ascontiguousarray(g1_whh[2 * H:].T), NJ, KH))
    w2in_a = _g4(_wtiles(np.ascontiguousarray(g2_wih[2 * H:].T), NJ, KH))

    def _group4(tiles, ng):
        """[nm, 128, KC*128] -> [ng, 128, 4*KC*128] (4 m-tiles per chunk)"""
        return np.ascontiguousarray(
            tiles.reshape(ng, 4, 128, KC * 128).transpose(0, 2, 1, 3)
            .reshape(ng, 128, 4 * KC * 128))

    wc_a = _group4(_wtiles(g1_wih[:, :C].T, MG, KC), 6)
    wini_a = _group4(_wtiles(np.asarray(fc_init_w, np.float32).T, MI, KC), 4)

    # prev-input weights: rz part as fp8 DR (x256), n part fp16 unscaled
    wp_t = g1_wih[:, C:].T                              # [130, 3072]
    wp8_a = np.zeros((128, MRZ, 2, 128), np.float32)
    wp8_a[:, :, 0] = (wp_t[:128, :MRZ * 128] * WS).reshape(128, MRZ, 128)
    wp8_a[:2, :, 1] = (wp_t[128:, :MRZ * 128] * WS).reshape(2, MRZ, 128)
    wp8_a = _q8(wp8_a)
    wp0n_a = np.ascontiguousarray(wp_t[:128, MRZ * 128:]).astype(f16)
    wp1n_a = np.zeros((128, NJ * 128), np.float32)
    wp1n_a[:2] = wp_t[128:, MRZ * 128:]
    wp1n_a = wp1n_a.astype(f16)

    wfco_a = np.zeros((128, KH, 256), np.float32)
    wfco_a[:, :, :D] = np.asarray(fco_w, np.float32).T.reshape(
        KH, 128, D).transpose(1, 0, 2)
    wfco_a = np.ascontiguousarray(wfco_a.reshape(128, KH * 256)).astype(f16)

    bias = np.zeros((128, NBIAS), np.float32)
    bias[:, B_INIT:B_INIT + MI] = _bias_cols(np.asarray(fc_init_b, np.float32), MI)
    bhh1 = np.asarray(g1_bhh, np.float32)
    bih1 = np.asarray(g1_bih, np.float32)
    bhh2 = np.asarray(g2_bhh, np.float32)
    bih2 = np.asarray(g2_bih, np.float32)
    bias[:, B_N1H:B_N1H + 8] = _bias_cols(bhh1[2 * H:], 8)
    bias[:, B_IH1:B_IH1 + 16] = _bias_cols((bih1 + bhh1)[:2 * H], 16) * WS
    bias[:, B_IH1 + 16:B_IH1 + 24] = _bias_cols(bih1[2 * H:], 8)
    bias[:, B_RZ2:B_RZ2 + 16] = _bias_cols(bih2[:2 * H] + bhh2[:2 * H], 16)
    bias[:, B_N2H:B_N2H + 8] = _bias_cols(bhh2[2 * H:], 8)
    bias[:, B_N2I:B_N2I + 8] = _bias_cols(bih2[2 * H:], 8)
    fco_b = np.asarray(fco_b, np.float32)
    bias[:, B_FCO] = fco_b[:128]
    bias[0:2, B_FCO + 1] = fco_b[128:130]

    prev_full = np.concatenate(
        [np.zeros((B, 1, D), np.float32), target[:, :T - 1]], axis=1)  # [B,T,D]

    in_maps = []
    for core in range(NCORES):
        e0 = core * EPC
        cf = c[e0:e0 + EPC].reshape(R, C)                  # [256, 512]
        cfT = np.ascontiguousarray(cf.T.reshape(KC, 128, R)).astype(f16)
        pv = prev_full[:, e0 * S:(e0 + EPC) * S]           # [B, 32, D]
        pv = pv.reshape(B, EPC, S, D).transpose(2, 1, 0, 3).reshape(S, R, D)
        pvT = np.ascontiguousarray(pv.transpose(0, 2, 1))  # [S, D, R]
        pvT1_pad = np.zeros((S, 128, R), np.float32)
        pvT1_pad[:, :2] = pvT[:, 128:130]
        pvT8 = np.zeros((S, 128, 2, R), np.float32)
        pvT8[:, :, 0] = pvT[:, :128]
        pvT8[:, :2, 1] = pvT[:, 128:130]
        in_maps.append({
            "cflatT": cfT,
            "prevT0": np.ascontiguousarray(pvT[:, :128]).astype(f16),
            "prevT1": pvT1_pad.astype(f16),
            "prevT8": _q8(pvT8),
            "w1h8": w1h8_a, "w2i8": w2i8_a, "w2h8": w2h8_a, "w2hn8": w2hn8_a,
            "w1hn": w1hn_a, "w2in": w2in_a,
            "wp8": wp8_a, "wp0n": wp0n_a, "wp1n": wp1n_a,
            "wc": wc_a, "wini": wini_a,
            "wfco": wfco_a, "biases": bias,
        })
    return in_maps


def assemble_output(results):
    """Per-core yT [S, 132, R] f32 -> full [B, T, D] f32."""
    T = E * S
    out = np.empty((B, T, D), np.float32)
    for core in range(NCORES):
        yt = results[core]["yT"]            # [S, 132, R]
        for ei in range(EPC):
            e = core * EPC + ei
            blk = yt[:, :D, ei * 128:(ei + 1) * 128]   # [S, D, 128]
            out[:, e * S:(e + 1) * S, :] = blk.transpose(2, 0, 1)
    return out


def kernel(c, target, length, batch_size, fc_init_w, fc_init_b,
           g1_wih, g1_whh, g1_bih, g1_bhh,
           g2_wih, g2_whh, g2_bih, g2_bhh, fco_w, fco_b):
    from concourse.bass_utils import run_bass_kernel_spmd

    if "nc" not in _cache:
        _cache["nc"] = build_program()
    nc = _cache["nc"]
    in_maps = prep_inputs(c, target, fc_init_w, fc_init_b,
                          g1_wih, g1_whh, g1_bih, g1_bhh,
                          g2_wih, g2_whh, g2_bih, g2_bhh, fco_w, fco_b)
    res = run_bass_kernel_spmd(nc, in_maps, list(range(NCORES)))
    return assemble_output(res.results)
